# revision 113
# baseline (speedup 1.0000x reference)
"""Trainium2 Bass kernel for nn_AutoregressiveDecoder (8-core data parallel).

Main ideas (159.4us baseline -> ~95.5us):
  - Macros whose rows never reach pass 2 fuse phase-1 into pass-1: the
    K=32 fp8 state extras accumulate onto the still-resident seq@w1 psum
    and gelu reads it directly (no fp8 base, no eviction, no eye re-add).
  - Mask-aware eval skipping: a step with round_mask=0 outputs pure ground
    truth and contributes no loss, so its eval is dead compute. Rows sort
    by mask popcount (desc), stripe across cores, and phase 2 runs as 3
    shrinking passes (all rows -> popcount>=2 -> popcount==3), cutting
    phase-2 work ~43% with zero numeric change. Host-known ground-truth
    state enters via a DMA'd initial state + per-pass predicated patches;
    masked-round outputs merge host-side.
  - fp8e4 DRSW matmuls for K=256-pair GEMMs; l1 eye-injection DR matmuls
    re-add the seq@w1 base while summing the K=15 state extras; l1 bias
    folded into the round-onehot rows.
  - Scalar engine (gelu ACT) is the steady-state bottleneck: 2-chunk PSUM
    tiles let each gelu batch 1024 free elems/instruction; when the l2
    biases are all zero (host-detected) the l2 gelus batch per tile too.
  - Phase-1 PSUM evictions split DVE (p-net) / ACT-copy (f-net, fills the
    scalar ramp-in idle); phase-1 emission interleaves with pass-1.
  - l3 packs pres output into M-row 2 of the fe accumulation group: one
    transpose instead of two; outputs stream per finishing region in the
    blocked on-chip layout (host de-blocks); final stragglers split so
    their serial chains pipeline; ACT tables preloaded in the preamble.
  - bf16 state tile for the transpose/extras path + f32 shadow for
    outputs; host recovers logits as log(p/(1-p)), losses in f64.
"""

import numpy as np
import ml_dtypes

import concourse.bass as bass
import concourse.bacc as bacc
import concourse.tile as tile
from concourse import mybir
from concourse.bass_utils import run_bass_kernel_spmd

BF16 = mybir.dt.bfloat16
F32 = mybir.dt.float32
FP8 = mybir.dt.float8e4
U8 = mybir.dt.uint8
AF = mybir.ActivationFunctionType
ALU = mybir.AluOpType
DR = mybir.MatmulPerfMode.DoubleRow
DRSW = mybir.MatmulPerfMode.DoubleRowSwInterleave
NP_BF16 = ml_dtypes.bfloat16
NP_F8 = ml_dtypes.float8_e4m3

B, D, H = 16384, 512, 512
NCORES = 8
S = 128.0           # fp8 weight scale (l2/l3)
S1 = 32.0           # l1 scale (base stored fp8 at S1x true)
GRAIN = 256         # row granularity (macro tail size)
ALL_PERMS = np.array(
    [[0, 1, 2], [0, 2, 1], [1, 0, 2], [1, 2, 0], [2, 0, 1], [2, 1, 0]], np.int32
)

# st slot map (32 slots per 32-row block, kind-major):
# 0:3 f(r0..r2), 3:6 p, 6:9 e, 9:12 flag, 12:15 roh, 15:32 zero
# pb slot map (8): 0 pfc, 1 sig, 2 pec, 3 one, 4:8 spare


def rv(t, s):
    """view a [32, n*s] tile as [32 p, n j, s slots]"""
    return t[:, :].rearrange("p (j s) -> p j s", s=s)


def build_graph(BLs, zb=False):
    """zb: host detected all-zero l2 biases -> batched bias-free gelus"""
    BL1 = BLs[0]
    assert all(b % GRAIN == 0 for b in BLs)
    nhalf = (BL1 % 512) // GRAIN
    nfull = BL1 // 512
    sizes = [512] * nfull + [256] * nhalf
    offs = np.cumsum([0] + sizes).tolist()
    NM = len(sizes)
    NP = len(BLs)

    nc = bacc.Bacc("TRN2", target_bir_lowering=False, debug=False,
                   num_devices=NCORES)

    # ---- dram parameters -------------------------------------------------
    seq_d = nc.dram_tensor("seq", [D, BL1], FP8, kind="ExternalInput").ap()
    sti_d = nc.dram_tensor("sti", [32, (BL1 // 32) * 32], BF16,
                           kind="ExternalInput").ap()
    # per-pass: u8 roh replicated x4 (pred-scatter predicate), bf16 roh
    mir_ds, rohb_ds, gm_ds, gv_ds = [], [], [], []
    for e, BLe in enumerate(BLs):
        blk = BLe // 32
        mir_ds.append(nc.dram_tensor(f"mir{e}", [32, blk * 12], U8,
                                     kind="ExternalInput").ap())
        rohb_ds.append(nc.dram_tensor(f"rohb{e}", [32, blk * 3], BF16,
                                      kind="ExternalInput").ap())
        if e > 0:  # state patch applied before pass e (gt of masked rounds)
            gm_ds.append(nc.dram_tensor(f"gm{e}", [32, blk * 12], U8,
                                        kind="ExternalInput").ap())
            gv_ds.append(nc.dram_tensor(f"gv{e}", [32, blk * 12], BF16,
                                        kind="ExternalInput").ap())

    # DRSW interleaved weight layouts: [128, (kpair, mchunk), 2*M]
    pw1_d = nc.dram_tensor("pw1", [128, 8 * 256], FP8, kind="ExternalInput").ap()
    fw1_d = nc.dram_tensor("fw1", [128, 8 * 256], FP8, kind="ExternalInput").ap()
    w1e_p_d = nc.dram_tensor("w1ep", [128, 1024], FP8, kind="ExternalInput").ap()
    w1e_f_d = nc.dram_tensor("w1ef", [128, 1024], FP8, kind="ExternalInput").ap()
    # plain K=32 extras weights for single-use macros (fused l1, no base)
    w1xp_d = nc.dram_tensor("w1xp8", [32, 512], FP8, kind="ExternalInput").ap()
    w1xf_d = nc.dram_tensor("w1xf8", [32, 512], FP8, kind="ExternalInput").ap()
    pw2_d = nc.dram_tensor("pw2", [128, 4 * 256], FP8, kind="ExternalInput").ap()
    pb2_d = nc.dram_tensor("pb2", [H // 2], F32, kind="ExternalInput").ap()
    fw2_d = nc.dram_tensor("fw2", [128, 8 * 256], FP8, kind="ExternalInput").ap()
    fb2_d = nc.dram_tensor("fb2", [H], F32, kind="ExternalInput").ap()
    pw3_d = nc.dram_tensor("pw3p", [H // 2, 32], FP8, kind="ExternalInput").ap()
    fw3_d = nc.dram_tensor("fw3p", [H, 32], FP8, kind="ExternalInput").ap()
    b3s_d = nc.dram_tensor("b3s", [1, 3], F32, kind="ExternalInput").ap()

    # outputs stay in the blocked on-chip layout; host de-blocks
    d9_d = nc.dram_tensor("d9", [32, (BL1 // 32) * 9], F32,
                          kind="ExternalOutput").ap()
    # raw S-scaled (pf, pe, logit) for macro-passes with no state update:
    # host finishes unscale/clip/sigmoid and demuxes by round
    raw_ds = [nc.dram_tensor(f"raw{e}", [3, BLe], F32,
                             kind="ExternalOutput").ap()
              for e, BLe in enumerate(BLs)]

    v = nc.vector
    sc = nc.scalar
    gp = nc.gpsimd
    te = nc.tensor

    with tile.TileContext(nc) as tc:
        wpool = tc.alloc_tile_pool(name="w", bufs=1)
        pers = tc.alloc_tile_pool(name="pers", bufs=1)
        bigp = tc.alloc_tile_pool(name="big", bufs=5)
        stp = tc.alloc_tile_pool(name="stp", bufs=3)
        smp = tc.alloc_tile_pool(name="smp", bufs=8)
        psp = tc.alloc_tile_pool(name="psum", bufs=3, space="PSUM")
        ps3 = tc.alloc_tile_pool(name="psum3", bufs=2, space="PSUM")

        # ---- load weights (persistent; split DMAs across queues so the
        # first phase-1 matmul's weights land fast) ----------------------
        pw1_sb = wpool.tile([128, 8, 256], FP8)
        fw1_sb = wpool.tile([128, 8, 256], FP8)
        pw1v = pw1_d[:, :].rearrange("p (i c) -> p i c", c=256)
        fw1v = fw1_d[:, :].rearrange("p (i c) -> p i c", c=256)
        for i in (0, 2, 1, 3):  # chunk order so macro-0's m=0,1 weights land first
            sc.dma_start(pw1_sb[:, 2 * i:2 * i + 2, :], pw1v[:, 2 * i:2 * i + 2, :])
            gp.dma_start(fw1_sb[:, 2 * i:2 * i + 2, :],
                         fw1v[:, 2 * i:2 * i + 2, :])
        # w1e rides gpsimd right after fw1 (needed ~15us); the scalar queue
        # stays short so the f-evictions aren't stuck behind DMA issues
        w1e_p_sb = wpool.tile([128, 8, 128], FP8)
        w1e_f_sb = wpool.tile([128, 8, 128], FP8)
        gp.dma_start(w1e_p_sb[:, :, :],
                     w1e_p_d[:, :].rearrange("p (i c) -> p i c", c=128))
        gp.dma_start(w1e_f_sb[:, :, :],
                     w1e_f_d[:, :].rearrange("p (i c) -> p i c", c=128))
        pw2_sb = wpool.tile([128, 4, 256], FP8)
        fw2_sb = wpool.tile([128, 8, 256], FP8)
        pw2v = pw2_d[:, :].rearrange("p (i c) -> p i c", c=256)
        fw2v = fw2_d[:, :].rearrange("p (i c) -> p i c", c=256)
        for i in range(2):
            gp.dma_start(pw2_sb[:, 2 * i:2 * i + 2, :], pw2v[:, 2 * i:2 * i + 2, :])
        for i in range(4):
            gp.dma_start(fw2_sb[:, 2 * i:2 * i + 2, :], fw2v[:, 2 * i:2 * i + 2, :])
        # single-use extras weights are needed only ~40us in: gpsimd tail
        w1xp_sb = wpool.tile([32, 512], FP8)
        w1xf_sb = wpool.tile([32, 512], FP8)
        gp.dma_start(w1xp_sb[:, :], w1xp_d[:, :])
        gp.dma_start(w1xf_sb[:, :], w1xf_d[:, :])
        pw3_sb = wpool.tile([128, 2, 32], FP8)
        fw3_sb = wpool.tile([128, 4, 32], FP8)
        for k in range(2):
            gp.dma_start(pw3_sb[:, k:k + 1, :], pw3_d[k * 128:(k + 1) * 128, :])
        for k in range(4):
            gp.dma_start(fw3_sb[:, k:k + 1, :], fw3_d[k * 128:(k + 1) * 128, :])
        if not zb:
            pb2_sb = wpool.tile([128, 2], F32)
            fb2_sb = wpool.tile([128, 4], F32)
            sc.dma_start(pb2_sb[:, :], pb2_d.rearrange("(m p) -> p m", p=128))
            sc.dma_start(fb2_sb[:, :], fb2_d.rearrange("(m p) -> p m", p=128))
        b3s_sb = wpool.tile([1, 3], F32)
        sc.dma_start(b3s_sb[:, :], b3s_d[:, :])
        b3bc = wpool.tile([32, 3], F32)
        gp.partition_broadcast(b3bc[:, :], b3s_sb[:, :])

        # hoist the ACT table loads into the idle preamble (a mid-stream
        # set switch stalls the scalar engine ~1.3us)
        dum = wpool.tile([1, 8], F32)
        v.memset(dum[:, :], 0.0)
        sc.activation(dum[0:1, 0:4], dum[0:1, 4:8], AF.Gelu)
        sc.activation(dum[0:1, 0:4], dum[0:1, 4:8], AF.Tanh, scale=0.5)

        # ---- phase 1: per-macro seq @ w1 bases (fp8 DRSW) ---------------
        base_p, base_f, sts, st9f = {}, {}, {}, {}
        ph1ps, seqTs = {}, {}
        # macros whose rows never reach pass 2 use their seq@w1 psum once:
        # fuse phase-1 into pass-1 (no fp8 base, no eviction, no eye re-add)
        single_use = lambda mt: NP < 2 or offs[mt] >= BLs[1]

        def phase1_evict(mt, on_scalar):
            """PSUM -> fp8 base evictions; delayed for far macros so they
            don't head-of-line block ready eval-pass work"""
            if single_use(mt):
                return
            nb = sizes[mt]
            for g in range(2):
                pp, pf_ = ph1ps[mt][g]
                v.tensor_copy(base_p[mt][:, 2 * g:2 * g + 2, 0:nb],
                              pp[:, :, 0:nb])
                if on_scalar:
                    sc.copy(base_f[mt][:, 2 * g:2 * g + 2, 0:nb],
                            pf_[:, :, 0:nb])
                else:
                    v.tensor_copy(base_f[mt][:, 2 * g:2 * g + 2, 0:nb],
                                  pf_[:, :, 0:nb])

        def phase1_macro(mt):
            nb = sizes[mt]
            nblk = nb // 32
            boff = offs[mt] // 32
            rows = slice(offs[mt], offs[mt] + nb)
            seqT = stp.tile([128, 4, 512], FP8, tag="seqT")
            for k in range(4):
                nc.sync.dma_start(seqT[:, k:k + 1, 0:nb],
                                  seq_d[k * 128:(k + 1) * 128, rows])
            if single_use(mt):
                seqTs[mt] = seqT  # consumed by the fused pass-1 l1
            else:
                base_p[mt] = pers.tile([128, 5, 512], FP8, tag=f"base_p{mt}",
                                       name=f"base_p{mt}")
                base_f[mt] = pers.tile([128, 5, 512], FP8, tag=f"base_f{mt}",
                                       name=f"base_f{mt}")
                # block 4 rows 15:128 multiply zero weights; must not be NaN
                v.memset(base_p[mt][:, 4:5, :], 0.0)
                v.memset(base_f[mt][:, 4:5, :], 0.0)
                ph1ps[mt] = {}
                for g in range(2):
                    pp = psp.tile([128, 2, 512], F32, tag="ps_main",
                                  name="ph1p")
                    pf_ = psp.tile([128, 2, 512], F32, tag="ps_main",
                                   name="ph1f")
                    ph1ps[mt][g] = (pp, pf_)
                    for m in (2 * g, 2 * g + 1):
                        j = m - 2 * g
                        for k in range(2):
                            te.matmul(pp[:, j:j + 1, 0:nb],
                                      pw1_sb[:, 4 * k + m:4 * k + m + 1, :],
                                      seqT[:, 2 * k:2 * k + 2, 0:nb],
                                      start=(k == 0), stop=(k == 1),
                                      perf_mode=DRSW)
                    for m in (2 * g, 2 * g + 1):
                        j = m - 2 * g
                        for k in range(2):
                            te.matmul(pf_[:, j:j + 1, 0:nb],
                                      fw1_sb[:, 4 * k + m:4 * k + m + 1, :],
                                      seqT[:, 2 * k:2 * k + 2, 0:nb],
                                      start=(k == 0), stop=(k == 1),
                                      perf_mode=DRSW)
            sts[mt] = pers.tile([32, nblk * 32], BF16, tag=f"st{mt}",
                                name=f"st{mt}")
            nc.sync.dma_start(sts[mt][:, :],
                              sti_d[:, boff * 32:(boff + nblk) * 32])
            st9f[mt] = pers.tile([32, nblk * 9], F32, tag=f"st9{mt}",
                                 name=f"st9{mt}")
            if not single_use(mt):
                v.memset(st9f[mt][:, :], 0.0)

        # ---- phase 2: shrinking eval passes, macro-interleaved ----------
        def pass_macro(e, mt, lo, hi, hook=None):
            """evaluate rows [lo, hi) of macro mt for pass e (32-aligned)"""
            nxt = BLs[e + 1] if e + 1 < NP else 0
            ns = hi - lo
            nsblk = ns // 32
            lob, hib = lo // 32, hi // 32
            boff = offs[mt] // 32
            # rows read again next pass need the state scatter (and with it
            # the full on-device plumbing); otherwise outputs ship raw
            hst_b = min(hib, max(nxt - offs[mt], 0) // 32)
            st3 = rv(sts[mt], 32)
            if hst_b > lob:
                mir_sb = smp.tile([32, 16 * 12], U8, tag="mir")
                nc.sync.dma_start(
                    mir_sb[:, 0:nsblk * 12],
                    mir_ds[e][:, (boff + lob) * 12:(boff + hib) * 12])
                m12 = rv(mir_sb, 12)
            rohb_sb = smp.tile([32, 16 * 3], BF16, tag="rohb")
            nc.sync.dma_start(
                rohb_sb[:, 0:nsblk * 3],
                rohb_ds[e][:, (boff + lob) * 3:(boff + hib) * 3])

            # pre-pass state patch: gt of masked rounds decoded since
            # this row's previous eval (host-known values)
            if e > 0:
                gm_sb = smp.tile([32, 16 * 12], U8, tag="gm")
                nc.sync.dma_start(
                    gm_sb[:, 0:nsblk * 12],
                    gm_ds[e - 1][:, (boff + lob) * 12:(boff + hib) * 12])
                gv_sb = smp.tile([32, 16 * 12], BF16, tag="gv")
                nc.sync.dma_start(
                    gv_sb[:, 0:nsblk * 12],
                    gv_ds[e - 1][:, (boff + lob) * 12:(boff + hib) * 12])
                v.copy_predicated(st3[:, lob:hib, 0:12],
                                  rv(gm_sb, 12)[:, 0:nsblk, :],
                                  rv(gv_sb, 12)[:, 0:nsblk, :])

            # state -> feature-major: roh into slots 12:15, transpose,
            # replicate to row strip 32 for the 2-way packed extras
            v.tensor_copy(st3[:, lob:hib, 12:15],
                          rv(rohb_sb, 3)[:, 0:nsblk, :])
            exT = smp.tile([32, 512], BF16, tag="exT")
            v.transpose(exT[0:32, 0:ns], sts[mt][:, lob * 32:hib * 32])
            if single_use(mt):
                ex8 = smp.tile([32, 512], FP8, tag="ex8")
                v.tensor_copy(ex8[0:32, 0:ns], exT[0:32, 0:ns])
            else:
                v.tensor_copy(base_p[mt][0:15, 4:5, lo:hi], exT[0:15, 0:ns])
                v.tensor_copy(base_f[mt][0:15, 4:5, lo:hi], exT[0:15, 0:ns])

            # layer 1: eye-injected base re-add + K=15 state extras in one
            # DR matmul per chunk (or, for single-use macros, seq@w1 DRSW
            # fused with a plain K=32 extras matmul); batched gelu -> fp8
            h1p = bigp.tile([128, 4, 512], FP8, tag="h1p")
            h1f = bigp.tile([128, 4, 512], FP8, tag="h1f")
            psl1 = {}
            for g in range(2):
                psl1[("p", g)] = psp.tile([128, 2, 512], F32,
                                          tag="ps_main", name="psl1p")
                psl1[("f", g)] = psp.tile([128, 2, 512], F32,
                                          tag="ps_main", name="psl1f")
            if single_use(mt):
                seqT = seqTs[mt]
                for g in range(2):
                    for net, wsb, wx in (("p", pw1_sb, w1xp_sb),
                                         ("f", fw1_sb, w1xf_sb)):
                        for m in (2 * g, 2 * g + 1):
                            j = m - 2 * g
                            out = psl1[(net, g)][:, j:j + 1, 0:ns]
                            for k in range(2):
                                te.matmul(out,
                                          wsb[:, 4 * k + m:4 * k + m + 1, :],
                                          seqT[:, 2 * k:2 * k + 2, lo:hi],
                                          start=(k == 0), stop=False,
                                          perf_mode=DRSW)
                            te.matmul(out, wx[:, m * 128:(m + 1) * 128],
                                      ex8[0:32, 0:ns],
                                      start=False, stop=True)
            else:
                for g in range(2):
                    for m in (2 * g, 2 * g + 1):
                        j = m - 2 * g
                        te.matmul(psl1[("p", g)][:, j:j + 1, 0:ns],
                                  w1e_p_sb[:, 2 * m:2 * m + 2, :],
                                  base_p[mt][:, m:5:(4 - m), lo:hi],
                                  start=True, stop=True, perf_mode=DR)
                    for m in (2 * g, 2 * g + 1):
                        j = m - 2 * g
                        te.matmul(psl1[("f", g)][:, j:j + 1, 0:ns],
                                  w1e_f_sb[:, 2 * m:2 * m + 2, :],
                                  base_f[mt][:, m:5:(4 - m), lo:hi],
                                  start=True, stop=True, perf_mode=DR)
            for g in range(2):
                sc.activation(h1p[:, 2 * g:2 * g + 2, 0:ns],
                              psl1[("p", g)][:, :, 0:ns],
                              AF.Gelu, scale=1.0 / S1)
                sc.activation(h1f[:, 2 * g:2 * g + 2, 0:ns],
                              psl1[("f", g)][:, :, 0:ns],
                              AF.Gelu, scale=1.0 / S1)
            if hook is not None:
                hook()

            # layer 2 (fp8 DRSW); per-chunk ACTs (bias differs per chunk)
            h2p = bigp.tile([128, 2, 512], FP8, tag="h2p")
            pp2 = psp.tile([128, 2, 512], F32, tag="ps_main", name="ps2p")
            for m in range(2):
                for k in range(2):
                    te.matmul(pp2[:, m:m + 1, 0:ns],
                              pw2_sb[:, 2 * k + m:2 * k + m + 1, :],
                              h1p[:, 2 * k:2 * k + 2, 0:ns],
                              start=(k == 0), stop=(k == 1),
                              perf_mode=DRSW)
            if zb:
                sc.activation(h2p[:, :, 0:ns], pp2[:, :, 0:ns],
                              AF.Gelu, scale=1.0 / S)
            else:
                for m in range(2):
                    sc.activation(h2p[:, m:m + 1, 0:ns],
                                  pp2[:, m:m + 1, 0:ns],
                                  AF.Gelu, scale=1.0 / S,
                                  bias=pb2_sb[:, m:m + 1])
            h2f = bigp.tile([128, 4, 512], FP8, tag="h2f")
            for g in range(2):
                pf2 = psp.tile([128, 2, 512], F32, tag="ps_main",
                               name="ps2f")
                for m in (2 * g, 2 * g + 1):
                    j = m - 2 * g
                    for k in range(2):
                        te.matmul(pf2[:, j:j + 1, 0:ns],
                                  fw2_sb[:, 4 * k + m:4 * k + m + 1, :],
                                  h1f[:, 2 * k:2 * k + 2, 0:ns],
                                  start=(k == 0), stop=(k == 1),
                                  perf_mode=DRSW)
                if zb:
                    sc.activation(h2f[:, 2 * g:2 * g + 2, 0:ns],
                                  pf2[:, :, 0:ns], AF.Gelu, scale=1.0 / S)
                else:
                    for m in (2 * g, 2 * g + 1):
                        j = m - 2 * g
                        sc.activation(h2f[:, m:m + 1, 0:ns],
                                      pf2[:, j:j + 1, 0:ns],
                                      AF.Gelu, scale=1.0 / S,
                                      bias=fb2_sb[:, m:m + 1])

            # layer 3: fe DR + pres (weights packed into M-row 2) share one
            # 32-row accumulation group -> single transpose
            p3 = ps3.tile([32, 512], F32, tag="ps3")
            for k in range(2):
                te.matmul(p3[:, 0:ns], fw3_sb[:, 2 * k:2 * k + 2, :],
                          h2f[:, 2 * k:2 * k + 2, 0:ns],
                          start=(k == 0), stop=False, perf_mode=DR)
            for k in range(2):
                te.matmul(p3[:, 0:ns], pw3_sb[:, k:k + 1, :],
                          h2p[:, k:k + 1, 0:ns],
                          start=False, stop=(k == 1))

            if hst_b <= lob:
                # no state update: ship raw feature-major predictions (one
                # small eviction; DMA can't read PSUM); the host finishes
                # unscale/clip/sigmoid and demuxes by round
                praw = smp.tile([3, 512], F32, tag="praw")
                v.tensor_copy(praw[0:3, 0:ns], p3[0:3, 0:ns])
                gp.dma_start(raw_ds[e][:, offs[mt] + lo:offs[mt] + hi],
                             praw[0:3, 0:ns])
            else:
                # back to blocked batch-major; unscale + bias
                feT = smp.tile([32, 16 * 32], F32, tag="feT")
                v.transpose(feT[:, 0:ns], p3[:, 0:ns])
                fe3 = rv(feT, 32)
                pf = fe3[:, 0:nsblk, 0:1]
                pe = fe3[:, 0:nsblk, 1:2]
                logit = fe3[:, 0:nsblk, 2:3]
                v.tensor_scalar(logit, logit, 1.0 / S, b3bc[:, 0:1],
                                ALU.mult, ALU.add)
                v.tensor_scalar(pf, pf, 1.0 / S, b3bc[:, 1:2],
                                ALU.mult, ALU.add)
                v.tensor_scalar(pe, pe, 1.0 / S, b3bc[:, 2:3],
                                ALU.mult, ALU.add)

                # candidates (all rows in an eval pass have mask=1, so the
                # predictions scatter directly; no gt merge needed)
                pb = smp.tile([32, 16 * 8], F32, tag="pb")
                pb8 = rv(pb, 8)[:, 0:nsblk, :]
                sig = pb8[:, :, 1:2]
                # sigmoid(l) = 0.5*tanh(0.5*l)+0.5 (stays on the gelu table)
                sc.activation(sig, logit, AF.Tanh, scale=0.5)
                v.tensor_scalar(sig, sig, 0.5, 0.5, ALU.mult, ALU.add)
                v.tensor_scalar(pb8[:, :, 0:1], pf, -10.0, 10.0,
                                ALU.max, ALU.min)
                v.tensor_scalar(pb8[:, :, 2:3], pe, -100.0, 100.0,
                                ALU.max, ALU.min)

                # scatter preds into round slots r, 3+r, 6+r (+flag) of the
                # bf16 state and the f32 output shadow
                v.memset(pb8[:, :, 3:4], 1.0)
                s9 = rv(st9f[mt], 9)
                for r in range(3):
                    v.copy_predicated(st3[:, lob:hst_b, r:r + 10:3],
                                      m12[:, 0:hst_b - lob, 4 * r:4 * r + 4],
                                      pb8[:, 0:hst_b - lob, 0:4])
                    v.copy_predicated(s9[:, lob:hib, r:r + 7:3],
                                      m12[:, 0:nsblk, 4 * r:4 * r + 3],
                                      pb8[:, :, 0:3])

            # s9 rows finishing at this pass stream out now (contiguous
            # blocked layout; host de-blocks); single-use macros' s9 was
            # never written (their only pred went out raw)
            fin_b = max(lob, (max(nxt - offs[mt], 0) + 31) // 32)
            if hib > fin_b and not (e == 0 and single_use(mt)):
                gp.dma_start(
                    d9_d[:, (boff + fin_b) * 9:(boff + hib) * 9],
                    st9f[mt][:, fin_b * 9:hib * 9])

        def emit_pass(e, mt, hook=None):
            nb = min(sizes[mt], BLs[e] - offs[mt])
            if nb <= 0:
                return
            if e == NP - 1 and NP > 1 and nb <= 256:
                # split the final straggler so its serial chain pipelines
                h = (nb // 64) * 32
                pass_macro(e, mt, 0, h, hook)
                pass_macro(e, mt, h, nb)
            elif e == NP - 2 and NP > 2 and offs[mt] + nb >= BLs[e] \
                    and nb == 512:
                # split the next-to-last pass's trailing macro too
                pass_macro(e, mt, 0, 256, hook)
                pass_macro(e, mt, 256, nb)
            else:
                pass_macro(e, mt, 0, nb, hook)

        # software-pipelined emission: 2-macro phase-1 lead, with far
        # macros' phase-1 work emitted AFTER the preceding eval-pass macro
        # so the PE queue prioritizes the matmuls the scalar engine needs
        for mt in range(min(2, NM)):
            phase1_macro(mt)
            phase1_evict(mt, on_scalar=True)
        for mt in range(NM):
            if mt + 2 < NM:
                phase1_macro(mt + 2)
                phase1_evict(mt + 2, on_scalar=True)
            emit_pass(0, mt)
        for e in range(1, NP):
            for mt in range(NM):
                emit_pass(e, mt)

        for p in (ps3, psp, smp, stp, bigp, pers, wpool):
            p.release()

    nc.compile()
    return nc


# ---------------------------------------------------------------------------
def prep_inputs(seq_embed, freq, pres, enrich,
                pw1, pb1, pw2, pb2, pw3, pb3,
                fw1, fb1, fw2, fb2, fw3, fb3,
                perm_idx, round_mask):
    """Host-side (numpy) popcount sort + sharding + pass schedule prep."""
    f32 = np.float32
    q8 = lambda a: np.ascontiguousarray(
        np.clip(np.asarray(a, f32), -240.0, 240.0).astype(NP_F8))
    bf = lambda a: np.ascontiguousarray(np.asarray(a, f32).astype(NP_BF16))

    rm = np.asarray(round_mask)
    pc_all = rm.sum(axis=1)
    act = np.flatnonzero(pc_all > 0)
    order = np.argsort(-pc_all[act], kind="stable")
    acts = act[order]
    core_rows = [acts[c::NCORES] for c in range(NCORES)]
    ceil_g = lambda n: max(GRAIN, -(-n // GRAIN) * GRAIN) if n > 0 else 0
    BL1 = ceil_g(max(len(r) for r in core_rows))
    n2 = max(int((pc_all[r] >= 2).sum()) for r in core_rows)
    n3 = max(int((pc_all[r] >= 3).sum()) for r in core_rows)
    BLs = [BL1] + [min(BL1, ceil_g(n)) for n in (n2, n3) if n > 0]

    pw1f, fw1f = np.asarray(pw1, f32), np.asarray(fw1, f32)
    pb1f, fb1f = np.asarray(pb1, f32), np.asarray(fb1, f32)

    # extras weights in the 15-slot basis (kind-major: f,p,e,flag,roh), x S,
    # with l1 bias folded into the roh rows
    w1x_p = np.zeros((15, H), f32)
    w1x_f = np.zeros((15, H), f32)
    for r in range(3):
        w1x_p[3 + r] = pw1f[512 + 2 * r]
        w1x_p[9 + r] = pw1f[513 + 2 * r]
        w1x_p[12 + r] = pw1f[518 + r] + pb1f
        w1x_f[0 + r] = fw1f[512 + 4 * r]
        w1x_f[3 + r] = fw1f[513 + 4 * r]
        w1x_f[6 + r] = fw1f[514 + 4 * r]
        w1x_f[9 + r] = fw1f[515 + 4 * r]
        w1x_f[12 + r] = fw1f[524 + r] + fb1f
    def w1e(w1x_n):
        """merged DR l1 weights [128, 8, 128]: pair (2m, 2m+1) = (eye,
        S1*w1x chunk m zero-padded to 128 rows)"""
        out = np.zeros((128, 8, 128), f32)
        for mm in range(4):
            out[:, 2 * mm, :] = np.eye(128, dtype=f32)
            out[0:15, 2 * mm + 1, :] = S1 * w1x_n[:, mm * 128:(mm + 1) * 128]
        return out.reshape(128, 1024)

    pw3p = np.zeros((256, 32), f32); pw3p[:, 2] = np.asarray(pw3, f32)[:, 0]
    fw3p = np.zeros((512, 32), f32); fw3p[:, 0:2] = np.asarray(fw3, f32)
    b3s = np.array([[np.asarray(pb3, f32)[0],
                     np.asarray(fb3, f32)[0], np.asarray(fb3, f32)[1]]], f32)

    def drsw(wm):
        """[K, M] -> [128, (K//256)*(M//128)*256] DRSW interleaved layout:
        per k-pair i and m-chunk mm, partition p holds
        [A[p,127], B[p,127], ..., A[p,0], B[p,0]] with A = rows 256i..+127,
        B = rows 256i+128..+255, cols of chunk mm reversed."""
        K, M = wm.shape
        KP, MC = K // 256, max(M // 128, 1)
        MW = min(M, 128)
        out = np.zeros((128, KP * MC, 2 * MW), f32)
        for i in range(KP):
            A = wm[256 * i:256 * i + 128, :]
            Bm = wm[256 * i + 128:256 * i + 256, :]
            for mm in range(MC):
                out[:, i * MC + mm, 0::2] = A[:, mm * MW:(mm + 1) * MW][:, ::-1]
                out[:, i * MC + mm, 1::2] = Bm[:, mm * MW:(mm + 1) * MW][:, ::-1]
        return out.reshape(128, KP * MC * 2 * MW)

    def w1x8(w1x_n):
        """plain K=32 extras weights [32, 512] for fused single-use l1"""
        out = np.zeros((32, H), f32)
        out[0:15] = S1 * w1x_n
        return q8(out)

    shared = {
        "pw1": q8(drsw(pw1f[:512] * S1)), "fw1": q8(drsw(fw1f[:512] * S1)),
        "w1ep": q8(w1e(w1x_p)), "w1ef": q8(w1e(w1x_f)),
        "w1xp8": w1x8(w1x_p), "w1xf8": w1x8(w1x_f),
        "pw2": q8(drsw(np.asarray(pw2, f32) * S)),
        "pb2": np.ascontiguousarray(np.asarray(pb2, f32)),
        "fw2": q8(drsw(np.asarray(fw2, f32) * S)),
        "fb2": np.ascontiguousarray(np.asarray(fb2, f32)),
        "pw3p": q8(pw3p * S), "fw3p": q8(fw3p * S),
        "b3s": b3s,
    }

    def blocked(a, k):
        """[BL, k] row-major -> [32, (BL//32)*k] blocked (b = 32*j + p)"""
        BL = a.shape[0]
        return np.ascontiguousarray(
            a.reshape(BL // 32, 32, k).transpose(1, 0, 2).reshape(32, -1))

    freq_f = np.asarray(freq, f32)
    pres_f = np.asarray(pres, f32)
    enr_f = np.asarray(enrich, f32)
    pidx = np.asarray(perm_idx)

    in_maps = []
    pass_rr = []
    for c in range(NCORES):
        rows = core_rows[c]
        n = len(rows)
        perm = np.zeros((BL1, 3), np.int64)
        ms = np.zeros((BL1, 3), np.int64)
        gt3 = np.zeros((BL1, 3, 3), f32)  # [row, step, (f,p,e)]
        perm[:n] = ALL_PERMS[pidx[rows]]
        ms[:n] = np.take_along_axis(rm[rows], perm[:n], 1)
        gt3[:n, :, 0] = np.take_along_axis(freq_f[rows], perm[:n], 1)
        gt3[:n, :, 1] = np.take_along_axis(pres_f[rows], perm[:n], 1)
        gt3[:n, :, 2] = np.take_along_axis(enr_f[rows], perm[:n], 1)
        cums = ms.cumsum(axis=1)
        ar = np.arange(BL1)
        # active step index per pass (sentinel 3 = none)
        se = []
        for e in range(len(BLs)):
            ex = cums[:, 2] >= e + 1
            s_e = np.where(ex, np.argmax((cums == e + 1) & (ms == 1), 1), 3)
            se.append(s_e)
        se.append(np.full(BL1, 3, np.int64))  # virtual pass after last

        def patch(lo, hi):
            """gt inject for masked steps t with lo < t < hi (elementwise)"""
            gvm = np.zeros((BL1, 12), f32)
            gmm = np.zeros((BL1, 12), np.uint8)
            for t in range(3):
                sel = np.flatnonzero((lo < t) & (t < hi) & (ms[:, t] == 0))
                r = perm[sel, t]
                gvm[sel, 0 + r] = gt3[sel, t, 0]
                gvm[sel, 3 + r] = gt3[sel, t, 1]
                gvm[sel, 6 + r] = gt3[sel, t, 2]
                gvm[sel, 9 + r] = 1.0
                gmm[sel, 0 + r] = 1
                gmm[sel, 3 + r] = 1
                gmm[sel, 6 + r] = 1
                gmm[sel, 9 + r] = 1
            return gvm, gmm

        im = dict(shared)
        core_rr = []
        seq = np.zeros((BL1, D), f32)
        seq[:n] = np.asarray(seq_embed, f32)[rows]
        im["seq"] = q8(seq.T)
        # initial state: gt of masked rounds decoded before first eval
        gvm, _ = patch(np.full(BL1, -1), se[0])
        sti = np.zeros((BL1, 32), f32)
        sti[:, 0:12] = gvm
        im["sti"] = bf(blocked(sti, 32))
        for e, BLe in enumerate(BLs):
            s_e = np.minimum(se[e], 2)
            valid = (se[e] < 3)
            core_rr.append((valid.copy(), perm[ar, s_e].copy()))
            roh = ((perm[ar, s_e][:, None] == np.arange(3)[None, :])
                   & valid[:, None]).astype(f32)
            im[f"mir{e}"] = np.ascontiguousarray(
                blocked(np.repeat(roh, 4, axis=1), 12)[:, :(BLe // 32) * 12]
                .astype(np.uint8))
            im[f"rohb{e}"] = bf(blocked(roh, 3)[:, :(BLe // 32) * 3])
            if e > 0:
                gvm, gmm = patch(se[e - 1], se[e])
                im[f"gm{e}"] = np.ascontiguousarray(
                    blocked(gmm, 12)[:, :(BLe // 32) * 12])
                im[f"gv{e}"] = bf(blocked(gvm, 12)[:, :(BLe // 32) * 12])
        in_maps.append(im)
        pass_rr.append(core_rr)

    zb = (not np.any(np.asarray(pb2, f32))) and (not np.any(np.asarray(fb2, f32)))
    aux = dict(core_rows=core_rows, pass_rr=pass_rr, BLs=list(BLs),
               b3=(float(np.asarray(pb3, f32)[0]),
                   float(np.asarray(fb3, f32)[0]),
                   float(np.asarray(fb3, f32)[1])),
               freq=freq_f, pres=pres_f, enrich=enr_f,
               round_mask=rm.astype(f32))
    return in_maps, aux, (tuple(BLs), zb)


def assemble(results, aux):
    """Merge device predictions (unmasked rounds only) over ground truth,
    then finish the loss reductions host-side in f64."""
    f32 = np.float32
    freq, pres, enrich = aux["freq"], aux["pres"], aux["enrich"]
    mr = aux["round_mask"]
    BLs = aux["BLs"]
    b3p, b3f, b3e = aux["b3"]
    # mirror the device macro layout to locate the raw (no-state) ranges
    BL1 = BLs[0]
    moffs = list(range(0, (BL1 // 512) * 512 + 1, 512))
    if BL1 % 512:
        moffs.append(BL1)
    df, dp, de = freq.copy(), pres.copy(), enrich.copy()
    for c, r in enumerate(results):
        rows = aux["core_rows"][c]
        n = len(rows)
        sel = mr[rows] > 0.5
        a = np.ascontiguousarray(r["d9"]).reshape(32, -1, 9)
        a = a.transpose(1, 0, 2).reshape(-1, 9)  # de-block: b = 32*j + p
        df[rows] = np.where(sel, a[:n, 0:3], freq[rows])
        dp[rows] = np.where(sel, a[:n, 3:6], pres[rows])
        de[rows] = np.where(sel, a[:n, 6:9], enrich[rows])
        # raw-path rows: finish unscale/clip/sigmoid here, demux by round
        for e, BLe in enumerate(BLs):
            nxt = BLs[e + 1] if e + 1 < len(BLs) else 0
            raw_lo = min([o for o in moffs if o >= nxt] + [BLe])
            if raw_lo >= BLe:
                continue
            valid, rr = aux["pass_rr"][c][e]
            ar = np.arange(len(valid))
            s = np.flatnonzero(valid & (ar >= raw_lo) & (ar < BLe))
            raw = np.asarray(r[f"raw{e}"], np.float64)
            idx, rd = rows[s], rr[s]
            df[idx, rd] = np.clip(raw[0, s] / S + b3f, -10.0, 10.0)
            dp[idx, rd] = 1.0 / (1.0 + np.exp(-(raw[2, s] / S + b3p)))
            de[idx, rd] = np.clip(raw[1, s] / S + b3e, -100.0, 100.0)
    lf = np.sum(np.square(df - freq) * mr, dtype=np.float64)
    le = np.sum(np.square(de - enrich) * mr, dtype=np.float64)
    p = np.clip(dp.astype(np.float64), 1e-12, 1.0 - 1e-12)
    lg = np.log(p / (1.0 - p))
    bce = np.maximum(lg, 0.0) - lg * pres + np.log1p(np.exp(-np.abs(lg)))
    lp = np.sum(bce * mr, dtype=np.float64)
    nm = np.sum(mr, dtype=np.float64) + 1e-8
    head = np.array([lf / nm, lp / nm, le / nm], f32)
    return np.concatenate([head, df.ravel(), dp.ravel(), de.ravel()])


_CACHE = {}


def _get_graph(key):
    if key not in _CACHE:
        _CACHE[key] = build_graph(key[0], zb=key[1])
    return _CACHE[key]


def _install_profile_hook():
    """Provide antenv.axon_hooks (missing in this image) so trace=True works."""
    import sys, types
    try:
        import antenv.axon_hooks  # noqa: F401
        return
    except ImportError:
        pass
    from trn_agent_boot.trn_boot import _ntff_profile_via_ctypes
    hook = _ntff_profile_via_ctypes('/opt/axon/libaxon_pjrt.so')
    mod = types.ModuleType('antenv.axon_hooks')
    mod._hook = hook
    mod.get_axon_ntff_profile_hook = lambda: mod._hook
    mod.set_axon_ntff_profile_hook = lambda h: setattr(mod, '_hook', h)
    sys.modules['antenv.axon_hooks'] = mod


def run(inputs, trace=False):
    if trace:
        _install_profile_hook()
    in_maps, aux, gkey = prep_inputs(**inputs)
    nc = _get_graph(gkey)
    res = run_bass_kernel_spmd(nc, in_maps, core_ids=list(range(NCORES)),
                               trace=trace)
    out = assemble(res.results, aux)
    return out, res


def kernel(**inputs):
    inputs = {k: np.asarray(v) for k, v in inputs.items()}
    out, _ = run(inputs)
    return out
